# revision 1
# baseline (speedup 1.0000x reference)
"""Dissipative Hamiltonian derivation — Trainium2 Bass kernel, 8-core SPMD.

Math (derived analytically from the jax reference; gradients computed in
closed form, no autodiff):
  vs = sigmoid(v); vq = [vs, q]; R = vq @ W1_w.T; U = R + b
  S[i,j] = ||r_i||^2 + ||u_j||^2 - 2 r_i.u_j          (= ||u_j - r_i||^2)
  dist = softplus(S); C = 2*mask*(dist^-2 - 2*dist^-3)*sigmoid(S)
  mask = (mvw*m).T @ (mvw*m)
  B[i] = (C @ U)[i] - rowsum(C)[i]*r_i        (local to the row shard)
  A[j] = colsum(C)[j]*u_j - (C.T @ R)[j]      (needs cross-core reduction)
  dHdq = (A - B) @ W1_w[:, 64:]
  (diagonal of C cancels exactly in A - B, so it is never zeroed)
  dq = dHdp = (2/m)*(softplus(zT)*sigmoid(zT)) @ W_T[:, 64:],  zT = [vs,p]@W_T.T
  dp = -(dHdq + (2/m)*(softplus(zF)*sigmoid(zF)) @ W_F),        zF = p@W_F.T

Sharding: rows of the N^2 pairwise computation, 192 rows per core.
Each core computes C for its 192 rows, reduces B locally, and contributes
P[j] = sum_{i in shard} c_ij*[r_i | 1] which is ReduceScatter-summed so
core c receives the slab of A-partials for its own 192 output rows.
"""

import os
import numpy as np

N = 1536
NCORES = 8
SH = N // NCORES            # 192 rows per core
H = 16
VD = 64
ITILES = [(0, 128), (128, 64)]   # i-tiles inside a shard (partition dim <= 128)
NJ = N // 128                # 12 j-chunks of 128
NJ3 = N // 512               # 3 j-chunks of 512

_CACHE = {}


def _build_nc():
    from concourse import bacc, mybir
    import concourse.tile as tile

    f32 = mybir.dt.float32
    AF = mybir.ActivationFunctionType

    # Bacc (not raw Bass): its compile() pipeline splits multi-sem waits
    # (move_matmul_waits_to_ldweights / generate_event_semaphores), which
    # TRN2 codegen requires for Tile-generated programs.
    nc = bacc.Bacc(None, num_devices=NCORES)

    def ein(name, shape):
        return nc.dram_tensor(name, shape, f32, kind="ExternalInput")

    vqT_d = ein("vqT", [96, N])       # [vs; q].T replicated
    vqTs_d = ein("vqTs", [96, SH])    # shard columns of vqT
    vpTs_d = ein("vpTs", [96, SH])    # [vs; p].T shard columns
    pTs_d = ein("pTs", [32, SH])
    m_d = ein("m_s", [SH, 1])
    mvwm_d = ein("mvwm", [48, N])     # mvw * m (mask factor), replicated
    mvwms_d = ein("mvwms", [48, SH])
    W1wT_d = ein("W1wT", [96, H])
    W1b_d = ein("W1b", [H, 1])
    W1q_d = ein("W1q", [H, 32])
    WTT_d = ein("WTT", [96, H])
    WTp_d = ein("WTp", [H, 32])
    WFT_d = ein("WFT", [32, H])
    WFm_d = ein("WFm", [H, 32])
    id_d = ein("ident", [128, 128])
    ones_d = ein("ones_row", [1, N])

    dp_d = nc.dram_tensor("dp_s", [SH, 32], f32, kind="ExternalOutput")
    dq_d = nc.dram_tensor("dq_s", [SH, 32], f32, kind="ExternalOutput")

    with tile.TileContext(nc) as tc:
        with (
            tc.tile_pool(name="const", bufs=1) as cp,
            tc.tile_pool(name="work", bufs=3) as wp,
            tc.tile_pool(name="dram", bufs=1, space="DRAM") as drp,
        ):
            def load(d, shape, tag):
                t = cp.tile(shape, f32, tag=tag)
                nc.sync.dma_start(t[:], d[:])
                return t

            vqT = load(vqT_d, [96, N], "vqT")
            vqTs = load(vqTs_d, [96, SH], "vqTs")
            vpTs = load(vpTs_d, [96, SH], "vpTs")
            pTs = load(pTs_d, [32, SH], "pTs")
            mvwm = load(mvwm_d, [48, N], "mvwm")
            mvwms = load(mvwms_d, [48, SH], "mvwms")
            W1wT = load(W1wT_d, [96, H], "W1wT")
            W1b = load(W1b_d, [H, 1], "W1b")
            W1q = load(W1q_d, [H, 32], "W1q")
            WTT = load(WTT_d, [96, H], "WTT")
            WTp = load(WTp_d, [H, 32], "WTp")
            WFT = load(WFT_d, [32, H], "WFT")
            WFm = load(WFm_d, [H, 32], "WFm")
            ident = load(id_d, [128, 128], "ident")

            UTx = cp.tile([H, N], f32, tag="UTx")      # U.T
            UTxX = cp.tile([2, N], f32, tag="UTxX")    # [ones; un2]
            ut2 = cp.tile([H, N], f32, tag="ut2")
            Slhs = cp.tile([H, SH], f32, tag="Slhs")   # -2 R.T
            SlhsX = cp.tile([2, SH], f32, tag="SlhsX")  # [rn2; ones]
            ones16 = cp.tile([H, 1], f32, tag="ones16")
            rts = cp.tile([H, SH], f32, tag="rts")     # R.T shard cols
            uts = cp.tile([H, SH], f32, tag="uts")     # U.T shard cols
            uro = cp.tile([128, 17 * NJ], f32, tag="uro")  # U rows | 1, per j-chunk
            rro0 = cp.tile([128, 17], f32, tag="rro0")     # R rows | 1, shard
            rro1 = cp.tile([64, 17], f32, tag="rro1")
            urs0 = cp.tile([128, H], f32, tag="urs0")      # U rows, shard
            urs1 = cp.tile([64, H], f32, tag="urs1")
            c0 = cp.tile([128, N], f32, tag="c0")
            c1 = cp.tile([64, N], f32, tag="c1")

            P_dram = drp.tile([N, 17], f32)
            P_red = drp.tile([SH, 17], f32)

            nc.vector.memset(ones16[:], 1.0)

            with tc.tile_pool(name="pss", bufs=4, space="PSUM") as pss:
                # U.T = R.T + b, full N
                for k in range(NJ3):
                    ps = pss.tile([H, 512], f32, tag="set")
                    nc.tensor.matmul(ps[:], W1wT[:], vqT[:, k * 512:(k + 1) * 512],
                                     start=True, stop=True)
                    nc.vector.tensor_scalar_add(UTx[:, k * 512:(k + 1) * 512],
                                                ps[:], W1b[:])
                # auxiliary rows [ones; un2] live in their own 2-partition tile
                # (DMA-written: partition 1 is off the quad boundary)
                nc.sync.dma_start(UTxX[0:1, :], ones_d[:, :])
                # un2 row = colwise ||u||^2
                nc.vector.tensor_mul(ut2[:], UTx[:], UTx[:])
                for k in range(NJ3):
                    ps = pss.tile([1, 512], f32, tag="set")
                    nc.tensor.matmul(ps[:], ones16[:], ut2[:, k * 512:(k + 1) * 512],
                                     start=True, stop=True)
                    tmp = wp.tile([1, 512], f32, tag="row")
                    nc.vector.tensor_copy(tmp[:], ps[:])
                    nc.sync.dma_start(UTxX[1:2, k * 512:(k + 1) * 512], tmp[:])
                # R.T shard cols
                ps = pss.tile([H, SH], f32, tag="set")
                nc.tensor.matmul(ps[:], W1wT[:], vqTs[:], start=True, stop=True)
                nc.vector.tensor_copy(rts[:], ps[:])
                nc.vector.tensor_scalar_add(uts[:], rts[:], W1b[:])
                # S lhsT main = -2 R.T ; aux rows = [rn2; ones]
                nc.vector.tensor_scalar_mul(Slhs[:], rts[:], -2.0)
                rts2 = wp.tile([H, SH], f32, tag="rts2")
                nc.vector.tensor_mul(rts2[:], rts[:], rts[:])
                ps = pss.tile([1, SH], f32, tag="set")
                nc.tensor.matmul(ps[:], ones16[:], rts2[:], start=True, stop=True)
                tmp = wp.tile([1, SH], f32, tag="row2")
                nc.vector.tensor_copy(tmp[:], ps[:])
                nc.sync.dma_start(SlhsX[0:1, :], tmp[:])
                nc.sync.dma_start(SlhsX[1:2, :], ones_d[:, 0:SH])
                # U rows (all N, by 128-chunk) and R/U rows for the shard
                for jc in range(NJ):
                    ps = pss.tile([128, H], f32, tag="tr")
                    nc.tensor.transpose(ps[:], UTx[:, jc * 128:(jc + 1) * 128],
                                        ident[0:H, 0:H])
                    nc.vector.tensor_copy(uro[:, jc * 17:jc * 17 + H], ps[:])
                    nc.vector.memset(uro[:, jc * 17 + H:jc * 17 + 17], 1.0)
                for it, (off, w) in enumerate(ITILES):
                    rro = (rro0, rro1)[it]
                    ps = pss.tile([w, H], f32, tag="tr")
                    nc.tensor.transpose(ps[:], rts[:, off:off + w], ident[0:H, 0:H])
                    nc.vector.tensor_copy(rro[:, 0:H], ps[:])
                    nc.vector.memset(rro[:, H:17], 1.0)
                    ps = pss.tile([w, H], f32, tag="tr")
                    nc.tensor.transpose(ps[:], uts[:, off:off + w], ident[0:H, 0:H])
                    nc.vector.tensor_copy((urs0, urs1)[it][:], ps[:])

            with (
                tc.tile_pool(name="psA", bufs=3, space="PSUM") as psA,
                tc.tile_pool(name="psB", bufs=2, space="PSUM") as psB,
                tc.tile_pool(name="psC", bufs=1, space="PSUM") as psC,
                tc.tile_pool(name="psD", bufs=2, space="PSUM") as psD,
            ):
                bsb = []
                for it, (off, w) in enumerate(ITILES):
                    ct = (c0, c1)[it]
                    for k in range(NJ3):
                        j0 = k * 512
                        sp = psA.tile([w, 512], f32, tag="sm")
                        nc.tensor.matmul(sp[:], Slhs[:, off:off + w],
                                         UTx[:, j0:j0 + 512], start=True, stop=False)
                        nc.tensor.matmul(sp[:], SlhsX[:, off:off + w],
                                         UTxX[:, j0:j0 + 512], start=False, stop=True)
                        mp = psA.tile([w, 512], f32, tag="sm")
                        nc.tensor.matmul(mp[:], mvwms[:, off:off + w],
                                         mvwm[:, j0:j0 + 512], start=True, stop=True)
                        sig = wp.tile([w, 512], f32, tag="sig")
                        nc.scalar.activation(sig[:], sp[:], AF.Sigmoid)
                        # softplus(S) = S + ln(1 + exp(-S)); exact identity,
                        # no overflow since S >= 0 (squared distance)
                        e1 = wp.tile([w, 512], f32, tag="e1")
                        nc.scalar.activation(e1[:], sp[:], AF.Exp, scale=-1.0)
                        l1 = wp.tile([w, 512], f32, tag="l1")
                        nc.scalar.activation(l1[:], e1[:], AF.Ln, bias=1.0)
                        dist = wp.tile([w, 512], f32, tag="dist")
                        nc.vector.tensor_add(dist[:], l1[:], sp[:])
                        # 2*(d^-2 - 2 d^-3) = 2*(d-2)*d^-3; the 2 is folded
                        # into the host-side mask factor. d^-3 = exp(-3 ln d)
                        # on ACT beats the iterative DVE reciprocal (~3.3us).
                        lnd = wp.tile([w, 512], f32, tag="lnd")
                        nc.scalar.activation(lnd[:], dist[:], AF.Ln)
                        p3 = wp.tile([w, 512], f32, tag="p3")
                        nc.scalar.activation(p3[:], lnd[:], AF.Exp, scale=-3.0)
                        t_ = wp.tile([w, 512], f32, tag="t_")
                        nc.vector.scalar_tensor_tensor(
                            t_[:], dist[:], -2.0, p3[:],
                            op0=mybir.AluOpType.add, op1=mybir.AluOpType.mult)
                        sm_ = wp.tile([w, 512], f32, tag="smt")
                        nc.vector.tensor_mul(sm_[:], sig[:], mp[:])
                        nc.vector.tensor_mul(ct[:, j0:j0 + 512], t_[:], sm_[:])
                    # B_part = C_shard @ [U | 1]  (transpose C chunks on PE)
                    bp = psC.tile([w, 17], f32, tag="acc")
                    for jc in range(NJ):
                        tp = psB.tile([128, w], f32, tag="ct")
                        nc.tensor.transpose(tp[:], ct[:, jc * 128:(jc + 1) * 128],
                                            ident[0:w, 0:w])
                        tsb = wp.tile([128, w], f32, tag="tsb")
                        nc.vector.tensor_copy(tsb[:], tp[:])
                        nc.tensor.matmul(bp[:], tsb[:], uro[:, jc * 17:(jc + 1) * 17],
                                         start=(jc == 0), stop=(jc == NJ - 1))
                    bs = wp.tile([w, 17], f32, tag="bsb")
                    nc.vector.tensor_copy(bs[:], bp[:])
                    bsb.append(bs)

                # P_part[j] = sum_{i in shard} c_ij * [r_i | 1]
                for jc in range(NJ):
                    pp = psD.tile([128, 17], f32, tag="p")
                    nc.tensor.matmul(pp[:], c0[:, jc * 128:(jc + 1) * 128], rro0[:],
                                     start=True, stop=False)
                    nc.tensor.matmul(pp[:], c1[:, jc * 128:(jc + 1) * 128], rro1[:],
                                     start=False, stop=True)
                    psb_ = wp.tile([128, 17], f32, tag="psb")
                    nc.vector.tensor_copy(psb_[:], pp[:])
                    nc.sync.dma_start(P_dram[jc * 128:(jc + 1) * 128, :], psb_[:])

                nc.gpsimd.collective_compute(
                    "ReduceScatter",
                    mybir.AluOpType.add,
                    replica_groups=[list(range(NCORES))],
                    ins=[P_dram.opt()],
                    outs=[P_red.opt()],
                )

                for it, (off, w) in enumerate(ITILES):
                    urs, rro, bs = (urs0, urs1)[it], (rro0, rro1)[it], bsb[it]
                    pr = wp.tile([w, 17], f32, tag="pr")
                    nc.sync.dma_start(pr[:], P_red[off:off + w, :])
                    # A = ccol*u - CtR ; B = CU - crow*r ; D = A - B
                    a_t = wp.tile([w, H], f32, tag="a_t")
                    nc.vector.tensor_scalar_mul(a_t[:], urs[:], pr[:, H:17])
                    nc.vector.tensor_sub(a_t[:], a_t[:], pr[:, 0:H])
                    b_t = wp.tile([w, H], f32, tag="b_t")
                    nc.vector.tensor_scalar_mul(b_t[:], rro[:, 0:H], bs[:, H:17])
                    d_t = wp.tile([w, H], f32, tag="d_t")
                    nc.vector.tensor_sub(d_t[:], bs[:, 0:H], b_t[:])
                    nc.vector.tensor_sub(d_t[:], a_t[:], d_t[:])
                    dtp = psB.tile([H, w], f32, tag="ct")
                    nc.tensor.transpose(dtp[:], d_t[:], ident[0:w, 0:w])
                    dts = wp.tile([H, w], f32, tag="dts")
                    nc.vector.tensor_copy(dts[:], dtp[:])
                    hq = psC.tile([w, 32], f32, tag="acc")
                    nc.tensor.matmul(hq[:], dts[:], W1q[:], start=True, stop=True)

                    # kinetic -> dq
                    m_t = wp.tile([w, 1], f32, tag="m_t")
                    nc.sync.dma_start(m_t[:], m_d[off:off + w, :])
                    mi2 = wp.tile([w, 1], f32, tag="mi2")
                    nc.vector.reciprocal(mi2[:], m_t[:])
                    nc.vector.tensor_scalar_mul(mi2[:], mi2[:], 2.0)
                    zt = psB.tile([w, H], f32, tag="ct")
                    nc.tensor.matmul(zt[:], vpTs[:, off:off + w], WTT[:],
                                     start=True, stop=True)
                    et = wp.tile([w, H], f32, tag="et")
                    nc.scalar.activation(et[:], zt[:], AF.Exp, scale=-1.0)
                    lt = wp.tile([w, H], f32, tag="lt")
                    nc.scalar.activation(lt[:], et[:], AF.Ln, bias=1.0)
                    pw = wp.tile([w, H], f32, tag="pw")
                    nc.vector.tensor_add(pw[:], lt[:], zt[:])
                    sg = wp.tile([w, H], f32, tag="sg")
                    nc.scalar.activation(sg[:], zt[:], AF.Sigmoid)
                    gz = wp.tile([w, H], f32, tag="gz")
                    nc.vector.tensor_mul(gz[:], pw[:], sg[:])
                    nc.vector.tensor_scalar_mul(gz[:], gz[:], mi2[:])
                    gtp = psB.tile([H, w], f32, tag="ct")
                    nc.tensor.transpose(gtp[:], gz[:], ident[0:w, 0:w])
                    gts = wp.tile([H, w], f32, tag="gts")
                    nc.vector.tensor_copy(gts[:], gtp[:])
                    dqp = psD.tile([w, 32], f32, tag="p")
                    nc.tensor.matmul(dqp[:], gts[:], WTp[:], start=True, stop=True)
                    dqs = wp.tile([w, 32], f32, tag="dqs")
                    nc.vector.tensor_copy(dqs[:], dqp[:])
                    nc.sync.dma_start(dq_d[off:off + w, :], dqs[:])

                    # dissipated -> dp
                    zf = psB.tile([w, H], f32, tag="ct")
                    nc.tensor.matmul(zf[:], pTs[:, off:off + w], WFT[:],
                                     start=True, stop=True)
                    ef = wp.tile([w, H], f32, tag="ef")
                    nc.scalar.activation(ef[:], zf[:], AF.Exp, scale=-1.0)
                    lf = wp.tile([w, H], f32, tag="lf")
                    nc.scalar.activation(lf[:], ef[:], AF.Ln, bias=1.0)
                    pwf = wp.tile([w, H], f32, tag="pwf")
                    nc.vector.tensor_add(pwf[:], lf[:], zf[:])
                    sgf = wp.tile([w, H], f32, tag="sgf")
                    nc.scalar.activation(sgf[:], zf[:], AF.Sigmoid)
                    gf = wp.tile([w, H], f32, tag="gf")
                    nc.vector.tensor_mul(gf[:], pwf[:], sgf[:])
                    nc.vector.tensor_scalar_mul(gf[:], gf[:], mi2[:])
                    gfp = psB.tile([H, w], f32, tag="ct")
                    nc.tensor.transpose(gfp[:], gf[:], ident[0:w, 0:w])
                    gfs = wp.tile([H, w], f32, tag="gfs")
                    nc.vector.tensor_copy(gfs[:], gfp[:])
                    ddp = psD.tile([w, 32], f32, tag="p")
                    nc.tensor.matmul(ddp[:], gfs[:], WFm[:], start=True, stop=True)
                    hqs = wp.tile([w, 32], f32, tag="hqs")
                    nc.vector.tensor_copy(hqs[:], hq[:])
                    dpsum = wp.tile([w, 32], f32, tag="dpsum")
                    nc.vector.tensor_add(dpsum[:], hqs[:], ddp[:])
                    dpo = wp.tile([w, 32], f32, tag="dpo")
                    nc.vector.tensor_scalar_mul(dpo[:], dpsum[:], -1.0)
                    nc.sync.dma_start(dp_d[off:off + w, :], dpo[:])

    nc.finalize()
    return nc


def _prepare_in_maps(v, e, m, p, q, mvw, W_T, W1_w, W1_b, W_F):
    f32 = np.float32
    v, m, p, q, mvw = (np.asarray(x, f32) for x in (v, m, p, q, mvw))
    W_T, W1_w, W1_b, W_F = (np.asarray(x, f32) for x in (W_T, W1_w, W1_b, W_F))

    vs = (1.0 / (1.0 + np.exp(-v))).astype(f32)
    vqT = np.ascontiguousarray(np.concatenate([vs, q], axis=1).T)    # [96,N]
    vpT = np.ascontiguousarray(np.concatenate([vs, p], axis=1).T)    # [96,N]
    pT = np.ascontiguousarray(p.T)                                   # [32,N]
    mvwm = np.ascontiguousarray(mvw * m[:, 0][None, :])              # [48,N]

    shared = {
        "vqT": vqT,
        "mvwm": mvwm,
        "W1wT": np.ascontiguousarray(W1_w.T),
        "W1b": np.ascontiguousarray(W1_b.reshape(H, 1)),
        "W1q": np.ascontiguousarray(W1_w[:, VD:]),
        "WTT": np.ascontiguousarray(W_T.T),
        "WTp": np.ascontiguousarray(W_T[:, VD:]),
        "WFT": np.ascontiguousarray(W_F.T),
        "WFm": np.ascontiguousarray(W_F),
        "ident": np.eye(128, dtype=f32),
        "ones_row": np.ones((1, N), dtype=f32),
    }
    in_maps = []
    for c in range(NCORES):
        sl = slice(c * SH, (c + 1) * SH)
        in_maps.append({
            **shared,
            "vqTs": np.ascontiguousarray(vqT[:, sl]),
            "vpTs": np.ascontiguousarray(vpT[:, sl]),
            "pTs": np.ascontiguousarray(pT[:, sl]),
            "m_s": np.ascontiguousarray(m[sl]),
            # factor 2 of the energy-derivative chain folded in here
            "mvwms": np.ascontiguousarray(2.0 * mvwm[:, sl]),
        })
    return in_maps


def kernel(v, e, m, p, q, mvw, W_T, W1_w, W1_b, W_F):
    from concourse.bass_utils import run_bass_kernel_spmd

    in_maps = _prepare_in_maps(v, e, m, p, q, mvw, W_T, W1_w, W1_b, W_F)

    if "nc" not in _CACHE:
        _CACHE["nc"] = _build_nc()
    nc = _CACHE["nc"]

    trace = bool(os.environ.get("BASS_KERNEL_TRACE"))
    if trace:
        try:
            from antenv.axon_hooks import get_axon_ntff_profile_hook  # noqa: F401
        except ImportError:
            trace = False
    res = run_bass_kernel_spmd(nc, in_maps, list(range(NCORES)), trace=trace)
    if trace and res.exec_time_ns is not None:
        print(f"HW exec time: {res.exec_time_ns} ns")

    dp = np.concatenate([res.results[c]["dp_s"] for c in range(NCORES)], axis=0)
    dq = np.concatenate([res.results[c]["dq_s"] for c in range(NCORES)], axis=0)
    return dp, dq



# revision 11
# speedup vs baseline: 1.5640x; 1.5640x over previous
"""Dissipative Hamiltonian derivation — Trainium2 Bass kernel, 8-core SPMD.

Math (closed-form gradients, identical derivation to the validated baseline):
  vs = sigmoid(v); vq = [vs, q]; R = vq @ W1_w.T; U = R + b
  S[i,j] = ||u_j - r_i||^2 ;  d = softplus(S)
  C[i,j] = 2*mask[i,j]*(d^-2 - 2 d^-3)*sigmoid(S),  mask = (mvw*m).T@(mvw*m)
  B[i] = (C @ [U|1])[i]      (row-local)
  P[j] = (C.T @ [R|1])[j]    (cross-core sum over row shards)
  A[j] = colsum(C)[j]*u_j - (C.T R)[j]
  dHdq = (A - B[:, :16] + rowsum*r) @ W1_w[:, 64:]   (diag of C cancels in A-B)
  dq = (2/m)*(softplus(zT)*sigmoid(zT)) @ W_T[:, 64:],  zT = [vs,p]@W_T.T
  dp = -(dHdq + (2/m)*(softplus(zF)*sigmoid(zF)) @ W_F), zF = p@W_F.T

Device program (per core, 192 rows of the N^2 computation):
  - S via one k=18 matmul per [w,512] tile (host packs [-2R.T; rn2; 1] / [U.T; 1; un2])
  - activation chain batched BY FUNCTION across all tiles (4 ACT table loads total)
  - C stored bf16; P = C.T@[R|1] per 128-col chunk -> DRAM -> AllToAll -> local sum
    (AllToAll floor ~5us on 8 cores vs ~31us measured for the RDH ReduceScatter)
  - B = C@[U|1] via PE transposes of bf16 C chunks, overlapped with the collective
  - kinetic/dissipated computed in transposed layout (zT.T = W_T @ [vs,p].T) so no
    PE transposes are needed there; their sigmoid/softplus ride the batched tables
"""

import os
import numpy as np

N = 1536
NCORES = 8
SH = N // NCORES            # 192 rows per core
H = 16
VD = 64
ITILES = [(0, 128), (128, 64)]   # i-tiles inside a shard (partition dim <= 128)
NJ = N // 128                # 12 j-chunks of 128
NJ3 = N // 512               # 3 j-chunks of 512

_CACHE = {}


def _build_nc():
    from concourse import bacc, mybir
    import concourse.tile as tile

    f32 = mybir.dt.float32
    bf16 = mybir.dt.bfloat16
    AF = mybir.ActivationFunctionType
    OP = mybir.AluOpType

    nc = bacc.Bacc(None, num_devices=NCORES)

    def ein(name, shape, dt=f32):
        return nc.dram_tensor(name, shape, dt, kind="ExternalInput")

    Srhs_d = ein("Srhs", [18, N])        # [U.T; ones; un2] replicated
    Slhs_d = ein("Slhs", [18, SH])       # [-2 R_s.T; rn2_s; ones]
    mvwm_d = ein("mvwm", [48, N])        # mvw * m, replicated
    mvwms2_d = ein("mvwms2", [48, SH])   # 2 * (mvw*m) shard cols
    uro_d = ein("uro", [128, 17 * NJ], bf16)   # [U|1] rows per 128-chunk
    rro0_d = ein("rro0", [128, 17], bf16)      # [R_s|1] rows
    rro1_d = ein("rro1", [64, 17], bf16)
    urs0_d = ein("urs0", [128, H])       # U_s rows fp32 (assembly)
    urs1_d = ein("urs1", [64, H])
    rrs0_d = ein("rrs0", [128, H])       # R_s rows fp32 (assembly)
    rrs1_d = ein("rrs1", [64, H])
    vpTs_d = ein("vpTs", [96, SH])       # [vs; p].T shard cols
    pTs_d = ein("pTs", [32, SH])
    mi2_d = ein("mi2", [SH, 1])          # 2/m (loaded as two <=128-row tiles)
    WTT_d = ein("WTT", [96, H])
    WTp_d = ein("WTp", [H, 32])
    WFT_d = ein("WFT", [32, H])
    WFm_d = ein("WFm", [H, 32])
    W1q_d = ein("W1q", [H, 32])
    identb_d = ein("identb", [128, 128], bf16)
    identf_d = ein("identf", [128, 128])

    dp_d = nc.dram_tensor("dp_s", [SH, 32], f32, kind="ExternalOutput")
    dq_d = nc.dram_tensor("dq_s", [SH, 32], f32, kind="ExternalOutput")

    with tile.TileContext(nc) as tc:
        with (
            tc.tile_pool(name="const", bufs=1) as cp,
            tc.tile_pool(name="work", bufs=3) as wp,
            tc.tile_pool(name="big", bufs=1) as wp1,
            tc.tile_pool(name="dram", bufs=1, space="DRAM") as drp,
        ):
            def load(d, shape, tag, dt=f32):
                t = cp.tile(shape, dt, tag=tag)
                nc.sync.dma_start(t[:], d[:])
                return t

            # loads in rough order of first use
            vpTs = load(vpTs_d, [96, SH], "vpTs")
            pTs = load(pTs_d, [32, SH], "pTs")
            WTT = load(WTT_d, [96, H], "WTT")
            WFT = load(WFT_d, [32, H], "WFT")
            Slhs = load(Slhs_d, [18, SH], "Slhs")
            Srhs = load(Srhs_d, [18, N], "Srhs")
            mvwms2 = load(mvwms2_d, [48, SH], "mvwms2")
            mvwm = load(mvwm_d, [48, N], "mvwm")
            rro0 = load(rro0_d, [128, 17], "rro0", bf16)
            rro1 = load(rro1_d, [64, 17], "rro1", bf16)
            identb = load(identb_d, [128, 128], "identb", bf16)
            uro = load(uro_d, [128, 17 * NJ], "uro", bf16)
            urs0 = load(urs0_d, [128, H], "urs0")
            urs1 = load(urs1_d, [64, H], "urs1")
            rrs0 = load(rrs0_d, [128, H], "rrs0")
            rrs1 = load(rrs1_d, [64, H], "rrs1")
            mi2_0 = cp.tile([128, 1], f32, tag="mi2_0")
            nc.sync.dma_start(mi2_0[:], mi2_d[0:128, :])
            mi2_1 = cp.tile([64, 1], f32, tag="mi2_1")
            nc.sync.dma_start(mi2_1[:], mi2_d[128:SH, :])
            WTp = load(WTp_d, [H, 32], "WTp")
            WFm = load(WFm_d, [H, 32], "WFm")
            W1q = load(W1q_d, [H, 32], "W1q")
            identf = load(identf_d, [128, 128], "identf")

            # C tiles, bf16, [i, j] layout
            ct0 = cp.tile([128, N], bf16, tag="ct0")
            ct1 = cp.tile([64, N], bf16, tag="ct1")

            P_dram = drp.tile([N, 17], f32)
            P_out = drp.tile([N, 17], f32)

            TILES = [(it, off, w, k) for k in range(NJ3)
                     for it, (off, w) in enumerate(ITILES)]

            # PSUM pools hand out full 2KB banks; sub-tiles are sliced out so
            # no extra banks are burned per distinct shape. 2+1+1+2+2 = 8 banks.
            with (
                tc.tile_pool(name="psS", bufs=2, space="PSUM") as psS,
                tc.tile_pool(name="psM", bufs=1, space="PSUM") as psM,
                tc.tile_pool(name="psP", bufs=1, space="PSUM") as psP,
                tc.tile_pool(name="psT", bufs=2, space="PSUM") as psT,
                tc.tile_pool(name="psB", bufs=2, space="PSUM") as psB,
            ):
                # ---- PE: kinetic/dissipated heads (transposed layout) ----
                zTb = psS.tile([128, 512], f32, tag="s")
                nc.tensor.matmul(zTb[0:H, 0:SH], WTT[:], vpTs[:],
                                 start=True, stop=True)
                zTt = wp.tile([H, SH], f32, tag="zTt")
                nc.vector.tensor_copy(zTt[:], zTb[0:H, 0:SH])
                zFb = psS.tile([128, 512], f32, tag="s")
                nc.tensor.matmul(zFb[0:H, 0:SH], WFT[:], pTs[:],
                                 start=True, stop=True)
                zFt = wp.tile([H, SH], f32, tag="zFt")
                nc.vector.tensor_copy(zFt[:], zFb[0:H, 0:SH])

                # ---- PE: S matmuls; copy S to SBUF so PSUM banks rotate ----
                S_sb = []
                for it, off, w, k in TILES:
                    sb_ = psS.tile([128, 512], f32, tag="s")
                    nc.tensor.matmul(sb_[0:w, 0:512], Slhs[:, off:off + w],
                                     Srhs[:, k * 512:(k + 1) * 512],
                                     start=True, stop=True)
                    ss = wp1.tile([w, 512], f32, tag=f"ss{it}{k}")
                    nc.vector.tensor_copy(ss[:], sb_[0:w, 0:512])
                    S_sb.append(ss)

                # ---- PE: mask matmuls (PSUM-resident until sm) ----
                mk_ps = []
                for it, off, w, k in TILES:
                    mb_ = psM.tile([128, 512], f32, tag="m")
                    nc.tensor.matmul(mb_[0:w, 0:512], mvwms2[:, off:off + w],
                                     mvwm[:, k * 512:(k + 1) * 512],
                                     start=True, stop=True)
                    mk_ps.append(mb_[0:w, 0:512])

                # ---- ACT batch 1: Sigmoid (kinetic rides the same table) ----
                sigT = wp.tile([H, SH], f32, tag="sigT")
                nc.scalar.activation(sigT[:], zTt[:], AF.Sigmoid)
                sigF = wp.tile([H, SH], f32, tag="sigF")
                nc.scalar.activation(sigF[:], zFt[:], AF.Sigmoid)
                sm_sb = []
                for n_, (it, off, w, k) in enumerate(TILES):
                    sg = wp1.tile([w, 512], f32, tag=f"sg{it}{k}")
                    nc.scalar.activation(sg[:], S_sb[n_][:], AF.Sigmoid)
                    # sm = sig * mask  (frees the mask PSUM bank)
                    sm = wp1.tile([w, 512], f32, tag=f"sm{it}{k}")
                    nc.vector.tensor_mul(sm[:], sg[:], mk_ps[n_])
                    sm_sb.append(sm)

                # ---- ACT batch 2 (natural_log_exp table, ONE load for all of
                # exp/ln below): softplus(x) = x + ln(1+exp(-x)), d^-3 =
                # exp(-3 ln d) ----
                eT = wp.tile([H, SH], f32, tag="eT")
                nc.scalar.activation(eT[:], zTt[:], AF.Exp, scale=-1.0)
                eF = wp.tile([H, SH], f32, tag="eF")
                nc.scalar.activation(eF[:], zFt[:], AF.Exp, scale=-1.0)
                e_sb = []
                for n_, (it, off, w, k) in enumerate(TILES):
                    ee = wp1.tile([w, 512], f32, tag=f"ee{it}{k}")
                    nc.scalar.activation(ee[:], S_sb[n_][:], AF.Exp, scale=-1.0)
                    e_sb.append(ee)
                lT = wp.tile([H, SH], f32, tag="lT")
                nc.scalar.activation(lT[:], eT[:], AF.Ln, bias=1.0)
                lF = wp.tile([H, SH], f32, tag="lF")
                nc.scalar.activation(lF[:], eF[:], AF.Ln, bias=1.0)
                l_sb = []
                for n_, (it, off, w, k) in enumerate(TILES):
                    ll = wp1.tile([w, 512], f32, tag=f"ll{it}{k}")
                    nc.scalar.activation(ll[:], e_sb[n_][:], AF.Ln, bias=1.0)
                    l_sb.append(ll)

                # ---- DVE: d = S + ln(1+exp(-S)); kinetic pw = z + ln(1+e^-z),
                # g = pw * sigmoid(z) (all off the ACT critical path) ----
                d_sb = []
                for n_, (it, off, w, k) in enumerate(TILES):
                    dd = wp1.tile([w, 512], f32, tag=f"dd{it}{k}")
                    nc.vector.tensor_add(dd[:], S_sb[n_][:], l_sb[n_][:])
                    d_sb.append(dd)
                pwT = wp.tile([H, SH], f32, tag="pwT")
                nc.vector.tensor_add(pwT[:], zTt[:], lT[:])
                gzT = wp.tile([H, SH], f32, tag="gzT")
                nc.vector.tensor_mul(gzT[:], pwT[:], sigT[:])
                pwF = wp.tile([H, SH], f32, tag="pwF")
                nc.vector.tensor_add(pwF[:], zFt[:], lF[:])
                gfT = wp.tile([H, SH], f32, tag="gfT")
                nc.vector.tensor_mul(gfT[:], pwF[:], sigF[:])

                # ---- ACT: lnd then d^-3 (same natural_log_exp table) ----
                ln_sb = []
                for n_, (it, off, w, k) in enumerate(TILES):
                    ld = wp1.tile([w, 512], f32, tag=f"ld{it}{k}")
                    nc.scalar.activation(ld[:], d_sb[n_][:], AF.Ln)
                    ln_sb.append(ld)
                p3_sb = []
                for n_, (it, off, w, k) in enumerate(TILES):
                    p3 = wp1.tile([w, 512], f32, tag=f"p3{it}{k}")
                    nc.scalar.activation(p3[:], ln_sb[n_][:], AF.Exp, scale=-3.0)
                    p3_sb.append(p3)

                # ---- DVE: C = ((d-2)*d^-3) * sm -> bf16; then per 512-chunk:
                # P = C.T@[R|1] (128-col pieces -> DRAM), B transposes+accum ----
                bb0 = psB.tile([128, 512], f32, tag="b")
                bb1 = psB.tile([128, 512], f32, tag="b")
                bp0 = bb0[0:128, 0:17]
                bp1 = bb1[0:64, 0:17]
                for k in range(NJ3):
                    for it, (off, w) in enumerate(ITILES):
                        n_ = 2 * k + it
                        ct = (ct0, ct1)[it]
                        t_ = wp.tile([w, 512], f32, tag=f"t{it}")
                        nc.vector.scalar_tensor_tensor(
                            t_[:], d_sb[n_][:], -2.0, p3_sb[n_][:],
                            op0=OP.add, op1=OP.mult)
                        nc.vector.tensor_mul(ct[:, k * 512:(k + 1) * 512],
                                             t_[:], sm_sb[n_][:])
                    for sub in range(4):
                        jc = 4 * k + sub
                        pb_ = psP.tile([128, 512], f32, tag="p")
                        pp = pb_[0:128, 0:17]
                        nc.tensor.matmul(pp, ct0[:, jc * 128:(jc + 1) * 128],
                                         rro0[:], start=True, stop=False)
                        nc.tensor.matmul(pp, ct1[:, jc * 128:(jc + 1) * 128],
                                         rro1[:], start=False, stop=True)
                        psb_ = wp.tile([128, 17], f32, tag="psb")
                        nc.vector.tensor_copy(psb_[:], pp)
                        nc.sync.dma_start(P_dram[jc * 128:(jc + 1) * 128, :],
                                          psb_[:])
                    for sub in range(4):
                        jc = 4 * k + sub
                        for it, (off, w) in enumerate(ITILES):
                            ct = (ct0, ct1)[it]
                            bp = (bp0, bp1)[it]
                            tb_ = psT.tile([128, 1024], bf16, tag="ct")
                            tp = tb_[0:128, 0:w]
                            nc.tensor.transpose(
                                tp, ct[:, jc * 128:(jc + 1) * 128],
                                identb[0:w, 0:w])
                            tsbb = wp.tile([128, w], bf16, tag=f"tsbb{it}")
                            nc.vector.tensor_copy(tsbb[:], tp)
                            nc.tensor.matmul(bp, tsbb[:],
                                             uro[:, jc * 17:(jc + 1) * 17],
                                             start=(jc == 0),
                                             stop=(jc == NJ - 1))

                nc.gpsimd.collective_compute(
                    "AllToAll",
                    mybir.AluOpType.bypass,
                    replica_groups=[list(range(NCORES))],
                    ins=[P_dram.opt()],
                    outs=[P_out.opt()],
                )

                bsb = []
                for it, (off, w) in enumerate(ITILES):
                    bs = wp.tile([w, 17], f32, tag=f"bsb{it}")
                    nc.vector.tensor_copy(bs[:], (bp0, bp1)[it])
                    bsb.append(bs)

                # ---- dq out (kinetic), dissipated held for dp ----
                ddp_sb = []
                for it, (off, w) in enumerate(ITILES):
                    qb_ = psP.tile([128, 512], f32, tag="p")
                    dqp = qb_[0:w, 0:32]
                    nc.tensor.matmul(dqp, gzT[:, off:off + w], WTp[:],
                                     start=True, stop=True)
                    mi2t = (mi2_0, mi2_1)[it]
                    dqs = wp.tile([w, 32], f32, tag="dqs")
                    nc.vector.tensor_scalar_mul(dqs[:], dqp, mi2t[:])
                    nc.sync.dma_start(dq_d[off:off + w, :], dqs[:])
                    fb_ = psP.tile([128, 512], f32, tag="p")
                    ddp = fb_[0:w, 0:32]
                    nc.tensor.matmul(ddp, gfT[:, off:off + w], WFm[:],
                                     start=True, stop=True)
                    dds = wp.tile([w, 32], f32, tag=f"dds{it}")
                    nc.vector.tensor_scalar_mul(dds[:], ddp, mi2t[:])
                    ddp_sb.append(dds)

                # ---- A2A result: load 8 slabs, tree-sum, assemble D, dp out ----
                for it, (off, w) in enumerate(ITILES):
                    acc = wp.tile([w, 8 * 17], f32, tag=f"acc{it}")
                    for c in range(NCORES):
                        nc.sync.dma_start(
                            acc[:, c * 17:(c + 1) * 17],
                            P_out[c * SH + off:c * SH + off + w, :])
                    for c in range(1, NCORES):
                        nc.vector.tensor_add(acc[:, 0:17], acc[:, 0:17],
                                             acc[:, c * 17:c * 17 + 17])
                    urs = (urs0, urs1)[it]
                    rrs = (rrs0, rrs1)[it]
                    bs = bsb[it]
                    # A - B = (urs*cc - P16) + (rrs*bc - bs16)
                    a_t = wp.tile([w, H], f32, tag="a_t")
                    nc.vector.scalar_tensor_tensor(
                        a_t[:], urs[:], acc[:, H:H + 1], acc[:, 0:H],
                        op0=OP.mult, op1=OP.subtract)
                    b_t = wp.tile([w, H], f32, tag="b_t")
                    nc.vector.scalar_tensor_tensor(
                        b_t[:], rrs[:], bs[:, H:17], bs[:, 0:H],
                        op0=OP.mult, op1=OP.subtract)
                    d_t = wp.tile([w, H], f32, tag="d_t")
                    nc.vector.tensor_add(d_t[:], a_t[:], b_t[:])
                    mb_ = psM.tile([128, 512], f32, tag="m")
                    dtp = mb_[0:H, 0:w]
                    nc.tensor.transpose(dtp, d_t[:], identf[0:w, 0:w])
                    dts = wp.tile([H, w], f32, tag="dts")
                    nc.vector.tensor_copy(dts[:], dtp)
                    hb_ = psP.tile([128, 512], f32, tag="p")
                    hq = hb_[0:w, 0:32]
                    nc.tensor.matmul(hq, dts[:], W1q[:], start=True, stop=True)
                    dpo = wp.tile([w, 32], f32, tag="dpo")
                    # dp = -(hq + ddp) = (hq * -1) - ddp
                    nc.vector.scalar_tensor_tensor(
                        dpo[:], hq, -1.0, ddp_sb[it][:],
                        op0=OP.mult, op1=OP.subtract)
                    nc.sync.dma_start(dp_d[off:off + w, :], dpo[:])

    nc.finalize()
    return nc


def _prepare_in_maps(v, e, m, p, q, mvw, W_T, W1_w, W1_b, W_F):
    f32 = np.float32
    v, m, p, q, mvw = (np.asarray(x, f32) for x in (v, m, p, q, mvw))
    W_T, W1_w, W1_b, W_F = (np.asarray(x, f32) for x in (W_T, W1_w, W1_b, W_F))
    bf16 = None
    import ml_dtypes
    bf16 = ml_dtypes.bfloat16

    vs = (1.0 / (1.0 + np.exp(-v))).astype(f32)
    vq = np.concatenate([vs, q], axis=1)                    # [N,96]
    R = (vq @ W1_w.T).astype(f32)                           # [N,16]
    U = (R + W1_b[None, :]).astype(f32)
    rn2 = (R * R).sum(axis=1).astype(f32)
    un2 = (U * U).sum(axis=1).astype(f32)
    ones = np.ones((N,), f32)

    Srhs = np.ascontiguousarray(np.vstack([U.T, ones[None, :], un2[None, :]]))
    Slhs_full = np.vstack([-2.0 * R.T, rn2[None, :], ones[None, :]])

    vpT = np.ascontiguousarray(np.concatenate([vs, p], axis=1).T)    # [96,N]
    pT = np.ascontiguousarray(p.T)                                   # [32,N]
    mvwm = np.ascontiguousarray(mvw * m[:, 0][None, :])              # [48,N]

    uro = np.empty((128, 17 * NJ), f32)
    for jc in range(NJ):
        uro[:, jc * 17:jc * 17 + H] = U[jc * 128:(jc + 1) * 128, :]
        uro[:, jc * 17 + H] = 1.0

    shared = {
        "Srhs": Srhs,
        "mvwm": mvwm,
        "uro": uro.astype(bf16),
        "WTT": np.ascontiguousarray(W_T.T),
        "WTp": np.ascontiguousarray(W_T[:, VD:]),
        "WFT": np.ascontiguousarray(W_F.T),
        "WFm": np.ascontiguousarray(W_F),
        "W1q": np.ascontiguousarray(W1_w[:, VD:]),
        "identb": np.eye(128, dtype=f32).astype(bf16),
        "identf": np.eye(128, dtype=f32),
    }
    in_maps = []
    for c in range(NCORES):
        sl = slice(c * SH, (c + 1) * SH)
        Rs, Us = R[sl], U[sl]
        rro = np.empty((SH, 17), f32)
        rro[:, 0:H] = Rs
        rro[:, H] = 1.0
        in_maps.append({
            **shared,
            "Slhs": np.ascontiguousarray(Slhs_full[:, sl]),
            "mvwms2": np.ascontiguousarray(2.0 * mvwm[:, sl]),
            "rro0": np.ascontiguousarray(rro[0:128]).astype(bf16),
            "rro1": np.ascontiguousarray(rro[128:]).astype(bf16),
            "urs0": np.ascontiguousarray(Us[0:128]),
            "urs1": np.ascontiguousarray(Us[128:]),
            "rrs0": np.ascontiguousarray(Rs[0:128]),
            "rrs1": np.ascontiguousarray(Rs[128:]),
            "vpTs": np.ascontiguousarray(vpT[:, sl]),
            "pTs": np.ascontiguousarray(pT[:, sl]),
            "mi2": np.ascontiguousarray(2.0 / m[sl]),
        })
    return in_maps


def kernel(v, e, m, p, q, mvw, W_T, W1_w, W1_b, W_F):
    from concourse.bass_utils import run_bass_kernel_spmd

    in_maps = _prepare_in_maps(v, e, m, p, q, mvw, W_T, W1_w, W1_b, W_F)

    if "nc" not in _CACHE:
        _CACHE["nc"] = _build_nc()
    nc = _CACHE["nc"]

    trace = bool(os.environ.get("BASS_KERNEL_TRACE"))
    if trace:
        try:
            from antenv.axon_hooks import get_axon_ntff_profile_hook  # noqa: F401
        except ImportError:
            trace = False
    res = run_bass_kernel_spmd(nc, in_maps, list(range(NCORES)), trace=trace)
    if trace and res.exec_time_ns is not None:
        print(f"HW exec time: {res.exec_time_ns} ns")

    dp = np.concatenate([res.results[c]["dp_s"] for c in range(NCORES)], axis=0)
    dq = np.concatenate([res.results[c]["dq_s"] for c in range(NCORES)], axis=0)
    return dp, dq
